# revision 9
# baseline (speedup 1.0000x reference)
"""NetVLAD Trainium2 kernel (Bass/Tile), data-parallel over batch on 8 cores.

Math (per batch b):
    x_hat = x / ||x||_2(channel)                    (B, D, H*W), D=512, N=1200
    logits = conv_w @ x_hat                         (K, N), K=64
    a = softmax_K(logits)
    vlad[k,d] = sum_n a[k,n] * x_hat[d,n] - (sum_n a[k,n]) * c[k,d]
    vlad = l2norm_rows(vlad); out = l2norm(flatten(vlad))   # == vlad_rows/8

v2 design notes (instruction-count-driven; the PE pays ~100-200ns per matmul
instruction regardless of size, so fewer/wider matmuls win):
  - ONE fused matmul per (n-chunk j, d-chunk a): stationary = x-chunk,
    moving = [identity | conv_wT_a | zeros], >=256 cols so fp32r streams at
    1 cycle/row.  The output region holds the transposed x chunk AND a
    logits partial.  d-chunk PAIRS share one 320-col PSUM span with the
    logits partial columns coinciding: the even chunk (start=True, 320
    moving cols incl. a zero tail) clears the whole span, the odd chunk
    (start=False, 256 cols) accumulates - its zero columns land on the
    even chunk's x^T, its x^T lands on the zero tail.  10 n-chunks only
    need 2 logits partials summed per chunk instead of 4.
  - x^T drains to SBUF as bf16 (ACT copy); sum(x^2) is a bf16 2x
    scalar_tensor_tensor split between DVE and ACT.
  - softmax runs per half-batch (chunks 0-4 / 5-9) so the aggregation for
    batch b can inject into batch b+1's fused stream early.
  - aggregation per chunk: 2 matmuls (256 + 258 cols) off one stationary
    a' load; cols 512:513 of the x^T tile carry s = ||x_n|| so the second
    matmul's last columns produce asum = sum_n a.
  - PSUM: F = [128, 3, 2, 512] (3 rotating chunk buffers, 2 banks each) +
    vl = [64, 2, 512] (2 banks) = 8 banks exactly.
  - Matmult instructions can carry only ONE sync wait (S3_LW), so tiny
    2x2 "absorber" matmuls observe semaphores one at a time (same trick
    as the warm matmul absorbing the x-DMA semaphore).
"""

import numpy as np

import concourse.bass as bass
import concourse.mybir as mybir
from concourse import bacc
import concourse.tile as tile
from concourse.bass_utils import run_bass_kernel_spmd
from concourse.masks import make_identity
from concourse.tile_rust import add_dep_helper

F32 = mybir.dt.float32
F32R = mybir.dt.float32r
BF16 = mybir.dt.bfloat16
ALU = mybir.AluOpType
ACTF = mybir.ActivationFunctionType
AX = mybir.AxisListType

P = 128
BPC = 8            # batches per core
D = 512
N = 1200
K = 64
DCH = D // P       # 4 d-chunks
NCHUNKS = [(j * P, min(P, N - j * P)) for j in range((N + P - 1) // P)]  # 10
NJ = len(NCHUNKS)
XW = 516           # xts row stride (512 d cols + 2 s cols + 2 pad), bf16
LN_EIGHTH = float(np.log(0.125))

# which engine does each chunk's sum(x^2): 'a' = ScalarE, 'd' = DVE
SUMSQ_ENG = ["d", "d", "d", "d", "d", "d", "d", "d", "a", "a"]
HALVES = [(0, 5), (5, NJ)]


def _emit(nc):
    x = nc.dram_tensor("x", (BPC, D, N), F32R, kind="ExternalInput")
    wt = nc.dram_tensor("wt", (D, K), F32R, kind="ExternalInput")
    cent = nc.dram_tensor("cent", (K, D), F32, kind="ExternalInput")
    out = nc.dram_tensor("out", (BPC, K, D), F32, kind="ExternalOutput")

    with tile.TileContext(nc) as tc:
        with (
            tc.tile_pool(name="const", bufs=1) as const,
            tc.tile_pool(name="xnat", bufs=3) as xnat_pool,
            tc.tile_pool(name="xtsb", bufs=2) as xt_pool,
            tc.tile_pool(name="softmax", bufs=2) as sm_pool,
            tc.tile_pool(name="smalls", bufs=2) as smalls,
            tc.tile_pool(name="scratch", bufs=1) as scratch,
            tc.tile_pool(name="epilog", bufs=2) as ep_pool,
            tc.tile_pool(name="psum", bufs=1, space="PSUM") as psum,
        ):
            identf = const.tile([P, P], F32)
            make_identity(nc, identf)
            # moving operands per d-chunk pair:
            #   even a: [ident(128) | wt_a(64) | zeros(128)]      -> 320 cols
            #   odd  a: [zeros(64) | wt_a(64) | ident(128)] (+64 unused)
            mov = const.tile([P, DCH, 320], F32R)
            zt = const.tile([P, 1, P], F32)
            nc.vector.memset(zt, 0.0)
            nc.vector.tensor_copy(
                mov[:, :, :].rearrange("p a (u c) -> p (a u) c", c=64),
                zt[:, :, 0:64].to_broadcast((P, DCH * 5, 64)),
            )
            wtr = wt.rearrange("(p a) k -> p a k", a=DCH)
            for a in range(DCH):
                icol = 0 if a % 2 == 0 else 128
                nc.vector.tensor_copy(mov[:, a, icol : icol + 128], identf)
                wcol = 128 if a % 2 == 0 else 64
                nc.sync.dma_start(mov[:, a, wcol : wcol + K], wtr[:, a, :])
            cent_sb = const.tile([K, D], F32)
            nc.sync.dma_start(cent_sb, cent[:, :])
            ln8 = const.tile([K, 1], F32)
            nc.vector.memset(ln8, LN_EIGHTH)

            # PSUM: immortal tensors, manually rotated.
            # F: 3 chunk buffers x 2 spans (d-chunk pairs) x 1 bank = 6 banks
            #    span layout: [xt_even(0:128) | lg(128:192) | xt_odd(192:320)]
            # vl: slab0 cols 0:256 = vlad[:, 0:256];
            #     slab1 cols 0:256 = vlad[:, 256:512], cols 256:258 = asum
            F = psum.tile([P, 3, 2, 512], F32)
            vl = psum.tile([K, 2, 512], F32)

            # PE pre-observes the gpsimd-produced identity so the first real
            # fused matmul carries a single sync wait.
            nc.tensor.transpose(F[0:P, 2, 1, 0:128], identf, identf)

            state = {}

            def absorber(tag, out_ap, read_ap, after=None):
                """2x2 matmul whose only job is to observe one semaphore."""
                m = nc.tensor.matmul(
                    out_ap,
                    read_ap,
                    read_ap,
                    start=True,
                    stop=True,
                    skip_group_check=True,
                )
                if after is not None:
                    add_dep_helper(m.ins, after.ins, sync=False, reason=tag)
                return m

            def sm_stage1(b, h):
                """ACT: sinv = rsqrt(ss) for chunks [h0, h1)."""
                h0, h1 = HALVES[h]
                t = state[b, "t"]
                ss, lss, sinv = t[5], t[10], t[11]
                nc.scalar.activation(lss[:, h0:h1], ss[:, h0:h1], ACTF.Ln)
                nc.scalar.activation(
                    sinv[:, h0:h1], lss[:, h0:h1], ACTF.Exp, scale=-0.5
                )

            def sm_stage2(b, h):
                """DVE: lgs = logits*sinv; ACT: e = exp(lgs)."""
                h0, h1 = HALVES[h]
                hn = h1 - h0
                t = state[b, "t"]
                lgc, lgs, expt, sinv = t[2], t[3], t[4], t[11]
                nc.vector.scalar_tensor_tensor(
                    out=lgs[:, h0:h1],
                    in0=lgc[:, h0:h1],
                    scalar=1.0,
                    in1=sinv[:, h0:h1, :].to_broadcast((P, hn, K)),
                    op0=ALU.mult,
                    op1=ALU.mult,
                )
                nc.scalar.activation(expt[:, h0:h1], lgs[:, h0:h1], ACTF.Exp)

            def sm_stage3a(b, h):
                """DVE: den = sum_k e, rden = 1/den."""
                h0, h1 = HALVES[h]
                t = state[b, "t"]
                expt, den, rden = t[4], t[6], t[7]
                nc.vector.tensor_reduce(
                    den[:, h0:h1], expt[:, h0:h1], axis=AX.X, op=ALU.add
                )
                nc.vector.reciprocal(rden[:, h0:h1], den[:, h0:h1])

            def sm_stage3b(b, h):
                """comb/s smalls, s-scatter (gpsimd), a' = e*comb."""
                h0, h1 = HALVES[h]
                hn = h1 - h0
                t = state[b, "t"]
                (xts, atp, _, _, expt, ss, _, rden, comb, s, _, sinv) = t
                nc.vector.tensor_tensor(
                    comb[:, h0:h1], rden[:, h0:h1], sinv[:, h0:h1], ALU.mult
                )
                nc.vector.tensor_tensor(
                    s[:, h0:h1], ss[:, h0:h1], sinv[:, h0:h1], ALU.mult
                )
                nc.gpsimd.tensor_copy(
                    xts[:, h0:h1, 512:514],
                    s[:, h0:h1, :].to_broadcast((P, hn, 2)),
                )
                nc.vector.scalar_tensor_tensor(
                    out=atp[:, h0:h1],
                    in0=expt[:, h0:h1],
                    scalar=1.0,
                    in1=comb[:, h0:h1, :].to_broadcast((P, hn, K)),
                    op0=ALU.mult,
                    op1=ALU.mult,
                )

            def softmax_half(b, h):
                sm_stage1(b, h)
                sm_stage2(b, h)
                sm_stage3a(b, h)
                sm_stage3b(b, h)

            def agg_chunks(b, cs):
                xts, atp = state[b][:2]
                for c in cs:
                    n0, cn = NCHUNKS[c]
                    if c == 0:
                        # one semaphore per absorber; see module docstring
                        absorber(
                            "agg: observe xts copy", F[0:2, 0, 0, 440:442],
                            xts[0:2, 0, 0:2],
                        )
                        absorber(
                            "agg: observe atp half1", F[0:2, 0, 0, 442:444],
                            atp[0:2, 0, 0:2],
                        )
                        # write INSIDE phase2(b-1)'s read region so the WAR
                        # dep lands here; overwritten by mm2(c=0) start=True
                        absorber(
                            "agg: observe vl WAR", vl[0:2, 1, 250:252],
                            atp[0:2, 0, 0:2],
                        )
                    if c == 5:
                        absorber(
                            "agg: observe atp half2", F[0:2, 0, 0, 444:446],
                            atp[0:2, 5, 0:2],
                        )
                    nc.tensor.matmul(
                        vl[:, 0, 0:256],
                        atp[:cn, c],
                        xts[:cn, c, 0:256],
                        start=(c == 0),
                        stop=(c == NJ - 1),
                        skip_group_check=True,
                    )
                    m2 = nc.tensor.matmul(
                        vl[:, 1, 0:258],
                        atp[:cn, c],
                        xts[:cn, c, 256:514],
                        start=(c == 0),
                        stop=(c == NJ - 1),
                        skip_group_check=True,
                    )
                    if c == NJ - 1:
                        state["last_pe"] = m2

            def phase1(b):
                xb = xnat_pool.tile([P, DCH, N], F32R, tag="xb")
                nc.sync.dma_start(
                    xb, x[b, :, :].rearrange("(p a) n -> p a n", a=DCH)
                )

                xts = xt_pool.tile([P, NJ, XW], BF16, tag="xts")
                atp = sm_pool.tile([P, NJ, K], BF16, tag="atp")
                lgc = sm_pool.tile([P, NJ, K], F32, tag="lgc")
                lgs = sm_pool.tile([P, NJ, K], BF16, tag="lgs")
                expt = sm_pool.tile([P, NJ, K], BF16, tag="expt")
                ss = smalls.tile([P, NJ, 1], F32, tag="ss")
                den = smalls.tile([P, NJ, 1], F32, tag="den")
                rden = smalls.tile([P, NJ, 1], F32, tag="rden")
                comb = smalls.tile([P, NJ, 1], F32, tag="comb")
                s = smalls.tile([P, NJ, 1], F32, tag="s")
                lss = smalls.tile([P, NJ, 1], F32, tag="lss")
                sinv = smalls.tile([P, NJ, 1], F32, tag="sinv")
                state[b] = (xts, atp)
                state[b, "t"] = (
                    xts, atp, lgc, lgs, expt, ss, den, rden, comb, s, lss, sinv
                )

                # warm matmul: absorbs the xb half-1 DMA semaphore; pinned
                # after the previous batch's last PE instruction so it cannot
                # hoist to where batch b-1 still drains F banks.
                warm = nc.tensor.matmul(
                    F[0:2, 0, 0, 446:448],
                    xb[:, 0, 0:2],
                    xb[:, 0, 0:2],
                    start=True,
                    stop=True,
                    skip_group_check=True,
                )
                if "last_pe" in state:
                    add_dep_helper(
                        warm.ins,
                        state["last_pe"].ins,
                        sync=False,
                        reason="pin warm after prior batch PE work",
                    )

                for j, (n0, nj) in enumerate(NCHUNKS):
                    jb = j % 3
                    # absorber for one of the two F-buffer WAR semaphores
                    # (ACT copy of chunk j-3); the first fused mm carries the
                    # other (DVE logits-reduce of chunk j-3).
                    ab = absorber(
                        f"F WAR j={j}", F[0:2, jb, 0, 120:122], xb[0:2, 0, 0:2],
                        after=warm if j == 0 else None,
                    )
                    for sp in range(2):
                        ae, ao = 2 * sp, 2 * sp + 1
                        m = nc.tensor.matmul(
                            F[:nj, jb, sp, 0:320],
                            xb[:, ae, n0 : n0 + nj],
                            mov[:, ae, 0:320],
                            start=True,
                            stop=False,
                            skip_group_check=True,
                        )
                        if sp == 0:
                            add_dep_helper(
                                m.ins, ab.ins, sync=False,
                                reason="absorber first",
                            )
                        nc.tensor.matmul(
                            F[:nj, jb, sp, 64:320],
                            xb[:, ao, n0 : n0 + nj],
                            mov[:, ao, 0:256],
                            start=False,
                            stop=True,
                            skip_group_check=True,
                        )
                    # drain: x^T chunk -> SBUF bf16.  x^T parts sit at
                    # span cols {0:128, 192:320}: AP [nj, 2, 2, 128].
                    cp_out = xts[:nj, j, 0:512].rearrange(
                        "p (s q c) -> p s q c", s=2, q=2
                    )
                    cp_in = F[:nj, jb, :, 0:384].rearrange(
                        "p s (q c) -> p s q c", q=2
                    )[:, :, :, 0:128]
                    if j % 2 == 0:
                        nc.scalar.activation(cp_out, cp_in, ACTF.Copy)
                    else:
                        nc.vector.tensor_copy(cp_out, cp_in)
                    # logits = sum of the 2 partials (DVE, strided reduce)
                    nc.vector.tensor_reduce(
                        lgc[:nj, j, :],
                        F[:nj, jb, :, 128:192].rearrange("p s c -> p c s"),
                        axis=AX.X,
                        op=ALU.add,
                    )
                    # ss[:, j] = sum(x^2) from the bf16 x^T copy
                    if j not in (4, 6, 8):
                        sq = scratch.tile([P, 512], BF16, tag="sqa")
                        nc.scalar.activation(
                            sq[:nj],
                            xts[:nj, j, 0:512],
                            ACTF.Square,
                            accum_out=ss[:nj, j, :],
                        )
                    else:
                        sq = scratch.tile([P, 512], BF16, tag="sqd")
                        nc.vector.scalar_tensor_tensor(
                            out=sq[:nj],
                            in0=xts[:nj, j, 0:512],
                            scalar=1.0,
                            in1=xts[:nj, j, 0:512],
                            op0=ALU.mult,
                            op1=ALU.mult,
                            accum_out=ss[:nj, j, :],
                        )
                    # batch-b softmax half-0, staged across chunk slots so
                    # no single DVE/ACT convoy starves the F-buffer drains
                    if j == 5:
                        sm_stage1(b, 0)
                    elif j == 6:
                        sm_stage2(b, 0)
                    elif j == 7:
                        sm_stage3a(b, 0)
                    elif j == 8:
                        sm_stage3b(b, 0)
                    # inject previous batch's aggregation (2 chunks/slot) and
                    # its epilogue, also staged
                    if b > 0:
                        prev = b - 1
                        if 2 <= j <= 6:
                            agg_chunks(prev, [2 * (j - 2), 2 * (j - 2) + 1])
                        elif j == 7:
                            phase2_a(prev)
                        elif j == 8:
                            phase2_b(prev)
                        elif j == 9:
                            phase2_c(prev)
                softmax_half(b, 1)

            def phase2_a(b):
                # negd = asum*c - vlad (negated; sign folded into the -1 in
                # phase2_c)
                negd = ep_pool.tile([K, D], BF16, tag="negd")
                state[b, "p2"] = negd
                asum = vl[:, 1, 256:257]
                nc.vector.scalar_tensor_tensor(
                    out=negd[:, 0:256],
                    in0=cent_sb[:, 0:256],
                    scalar=asum,
                    in1=vl[:, 0, 0:256],
                    op0=ALU.mult,
                    op1=ALU.subtract,
                )
                nc.vector.scalar_tensor_tensor(
                    out=negd[:, 256:512],
                    in0=cent_sb[:, 256:512],
                    scalar=asum,
                    in1=vl[:, 1, 0:256],
                    op0=ALU.mult,
                    op1=ALU.subtract,
                )

            def phase2_b(b):
                negd = state[b, "p2"]
                sqp = ep_pool.tile([K, D], BF16, tag="sqp")
                ssk = ep_pool.tile([K, 1], F32, tag="ssk")
                state[b, "p2b"] = ssk
                nc.vector.scalar_tensor_tensor(
                    out=sqp,
                    in0=negd,
                    scalar=1.0,
                    in1=negd,
                    op0=ALU.mult,
                    op1=ALU.mult,
                    accum_out=ssk,
                )
                lssk = ep_pool.tile([K, 1], F32, tag="lssk")
                state[b, "p2l"] = lssk
                nc.scalar.activation(lssk, ssk, ACTF.Ln)

            def phase2_c(b):
                negd = state.pop((b, "p2"))
                state.pop((b, "p2b"))
                lssk = state.pop((b, "p2l"))
                state.pop(b)
                state.pop((b, "t"))
                # gk = (1/8) * rsqrt(ssk) == exp(-0.5*ln(ssk) + ln(1/8))
                gk = ep_pool.tile([K, 1], F32, tag="gk")
                nc.scalar.activation(gk, lssk, ACTF.Exp, scale=-0.5, bias=ln8)
                ot = ep_pool.tile([K, D], F32, tag="ot")
                nc.vector.tensor_scalar(
                    out=ot,
                    in0=negd,
                    scalar1=gk,
                    scalar2=-1.0,
                    op0=ALU.mult,
                    op1=ALU.mult,
                )
                nc.sync.dma_start(out[b, :, :], ot)

            for b in range(BPC):
                phase1(b)
            agg_chunks(BPC - 1, list(range(NJ)))
            phase2_a(BPC - 1)
            phase2_b(BPC - 1)
            phase2_c(BPC - 1)

    return nc


_NC = None


def _patch_act_tables():
    """Force every ScalarE activation onto the one table set that contains
    {copy, square, ln, exp} so the kernel pays a single ACT_TABLE_LOAD."""
    import concourse.bacc as _bacc_mod
    orig = _bacc_mod.get_activation_tables

    def patched(arch):
        tables = dict(orig(arch))
        assert "natural_log_exp_and_others" in tables
        return {
            name: (funcs if name == "natural_log_exp_and_others" else set())
            for name, funcs in tables.items()
        }

    _bacc_mod.get_activation_tables = patched


def _get_nc():
    global _NC
    if _NC is None:
        _patch_act_tables()
        nc = bacc.Bacc("TRN2", target_bir_lowering=False)
        _emit(nc)
        nc.compile()
        _NC = nc
    return _NC


# xts column c (within a chunk) holds d = 4*(c%128) + c//128; centroids are
# pre-permuted to match and the kernel output is un-permuted on the host.
_DPERM = np.array([4 * (c % 128) + c // 128 for c in range(D)])


def _make_in_maps(x, conv_w, centroids):
    B = x.shape[0]
    xs = np.ascontiguousarray(x, dtype=np.float32).reshape(B, D, N)
    wt = np.ascontiguousarray(conv_w.T, dtype=np.float32)
    cent = np.ascontiguousarray(
        np.asarray(centroids, dtype=np.float32)[:, _DPERM]
    )
    in_maps = []
    for c in range(8):
        in_maps.append(
            {
                "x": np.ascontiguousarray(xs[c * BPC : (c + 1) * BPC]),
                "wt": wt,
                "cent": cent,
            }
        )
    return in_maps


def _run(x, conv_w, centroids, trace=False):
    nc = _get_nc()
    res = run_bass_kernel_spmd(
        nc,
        _make_in_maps(x, conv_w, centroids),
        core_ids=list(range(8)),
        trace=trace,
    )
    outs = [r["out"] for r in res.results]
    full = np.concatenate(outs, axis=0)  # (B, K, D), d-permuted
    unperm = np.empty_like(full)
    unperm[:, :, _DPERM] = full
    return unperm.reshape(-1, K * D), res


def kernel(x, conv_w, centroids):
    full, _ = _run(x, conv_w, centroids, trace=False)
    return full
